# revision 3
# baseline (speedup 1.0000x reference)
"""Trainium2 Bass kernel: multi-head self-attention with RoPE, causal mask.

Reference semantics (B=2, S=2048, D=1024, H=16, DK=64):
    q = rope(x @ Wq.T), k = rope(x @ Wk.T), v = x @ Wv.T   (per-head views)
    out = softmax(causal(q k^T / 8)) v ;  y = out @ Wo.T

Sharding over 8 cores: 2-way batch x 4-way heads (4 heads/core).
Each core computes a partial y [S, D] (its heads' contribution); host sums
the 4 partials per batch (device output is fp16, summed in fp64 on host).

On-device layout strategy (per core):
  - all 16-bit operands are fp16; host prepacks every weight/input so each
    SBUF tensor loads with ONE wide DMA (xT in 4 per-sg transfers)
  - input loads ride the scalar queue; SBUF-SBUF rope swaps and y
    writebacks ride sync (no head-of-line blocking between the streams)
  - stage A is emitted per-512-column group (sg): K ec0 + Q ec0 + four V
    chunks, so the PE streams as soon as each sg's xT slice lands
  - V is projected TRANSPOSED directly (lhsT = x chunk, rhs = Wv) into
    [s, e] PSUM tiles -- no PE transpose pass; one strided copy scatters
    all 4 heads into the V working layout (ones col 0 for the softmax
    denominator, data at cols 64..127)
  - attention is HEAD-PAIR-OUTER with the ec1 K/Q projection chunks and
    the out-projection interleaved into the kc streams as PE filler
  - scores are computed TRANSPOSED (k on partitions, q on free); both
    heads of a pair share ONE two-bank PSUM tile so a single Exp covers
    both; causal masks only touch the true 128-col diagonal sub-block
  - rope multiplies all on DVE (gpsimd only does masks + memsets)
  - normalization: reciprocal_approx_fast written straight into an f32r
    tile + PE broadcast matmul; multiply deferred into the next stream
"""

import sys

sys.path.insert(0, "/opt/trn_rl_repo")

import numpy as np


S = 2048
D = 1024
NH = 16
DK = 64
HL = 4          # heads per core
EL = HL * DK    # 256 local e-dims
N_CORES = 8
THETA = 10000.0

_compiled = None


def _build():
    import concourse.bacc as bacc
    import concourse.tile as tile
    from concourse import mybir
    from concourse.alu_op_type import AluOpType

    dt = mybir.dt
    f32, f32r = dt.float32, dt.float32r
    f16 = dt.float16

    nc = bacc.Bacc("TRN2", target_bir_lowering=False, debug=False,
                   num_devices=N_CORES)

    xt_d = nc.dram_tensor("xt", [4, 128, 8, 512], f16, kind="ExternalInput").ap()
    wq_d = nc.dram_tensor("wq", [128, 8 * EL], f16, kind="ExternalInput").ap()
    wk_d = nc.dram_tensor("wk", [128, 8 * EL], f16, kind="ExternalInput").ap()
    wv_d = nc.dram_tensor("wv", [128, 8 * EL], f16, kind="ExternalInput").ap()
    wo_d = nc.dram_tensor("wo", [128, 2 * D], f16, kind="ExternalInput").ap()
    cos_d = nc.dram_tensor("cosT", [128, S], f16, kind="ExternalInput").ap()
    sin_d = nc.dram_tensor("sinT", [128, S], f16, kind="ExternalInput").ap()
    y = nc.dram_tensor("y", [S, D], f16, kind="ExternalOutput").ap()

    with tile.TileContext(nc) as tc:
        with tc.tile_pool(name="persist", bufs=1) as pp:
            # persistent SBUF tiles (live across both stages)
            qt = [pp.tile([128, S], f16, tag=f"qt{c}", name=f"qt{c}") for c in range(2)]
            ktz = [[pp.tile([128, S], f16, tag=f"ktz{c}{par}", name=f"ktz{c}{par}")
                   for par in range(2)] for c in range(2)]
            vh_all = pp.tile([128, HL * 16 * 128], f16, tag="vh", name="vh")
            cos_sb = pp.tile([128, S], f16, tag="cos", name="cos")
            sin_sb = pp.tile([128, S], f16, tag="sin", name="sin")
            xt_all = pp.tile([128, 8 * S], f16, tag="xt", name="xt")
            wv_all = pp.tile([128, 8 * EL], f16, tag="wv", name="wv")
            wk_all = pp.tile([128, 8 * EL], f16, tag="wk", name="wk")
            wq_all = pp.tile([128, 8 * EL], f16, tag="wq", name="wq")
            wo_all = pp.tile([128, 2 * D], f16, tag="wo", name="wo")
            warm = pp.tile([128, 256], f16, tag="warm", name="warm")

            xtv = xt_all[:].rearrange("p (d s) -> p d s", d=8)
            wvv = wv_all[:].rearrange("p (d e) -> p d e", d=8)
            wkv = wk_all[:].rearrange("p (d e) -> p d e", d=8)
            wqv = wq_all[:].rearrange("p (d e) -> p d e", d=8)
            wov = wo_all[:].rearrange("p (c d) -> p c d", c=2)
            vhv = vh_all[:].rearrange("p (h s c) -> p h s c", h=HL, c=128)

            # ---- input DMA program ----
            # scalar queue: all HBM loads, in arrival-priority order.
            # sync queue: cos/sin first, then (as emitted) rope swaps + y.
            nc.sync.dma_start(cos_sb[:], cos_d[:])
            nc.sync.dma_start(sin_sb[:], sin_d[:])
            nc.scalar.dma_start(wv_all[:], wv_d[:])
            nc.scalar.dma_start(
                xtv[:, :, 0:512], xt_d[0])
            nc.scalar.dma_start(wk_all[:], wk_d[:])
            nc.scalar.dma_start(wq_all[:], wq_d[:])
            for sg in range(1, 4):
                nc.scalar.dma_start(
                    xtv[:, :, 512 * sg:512 * (sg + 1)], xt_d[sg])
            nc.scalar.dma_start(wo_all[:], wo_d[:])

            # rope chunk: evacuate PSUM proj, SBUF-to-SBUF DMA block swap to
            # build the rotate-half partner, cos/sin multiplies + add (DVE)
            def rope_chunk(ps, qa, qas, sg, is_k, ec, ropool, evac):
                sl = slice(512 * sg, 512 * (sg + 1))
                evac(qa[:, sl], ps[:])
                for blk in range(2):
                    b0 = 64 * blk
                    nc.sync.dma_start(
                        qas[b0:b0 + 32, sl], qa[b0 + 32:b0 + 64, sl])
                    nc.sync.dma_start(
                        qas[b0 + 32:b0 + 64, sl], qa[b0:b0 + 32, sl])
                qc = ropool.tile([128, 512], f16, tag="qc", name="qc")
                qs = ropool.tile([128, 512], f16, tag="qs", name="qs")
                nc.vector.tensor_mul(qc[:], qa[:, sl], cos_sb[:, sl])
                nc.vector.tensor_mul(qs[:], qas[:, sl], sin_sb[:, sl])
                if is_k:
                    # zero-padded K halves so score matmuls see K=128 rows
                    nc.vector.tensor_add(
                        ktz[ec][0][0:64, sl], qc[0:64, :], qs[0:64, :])
                    nc.vector.tensor_add(
                        ktz[ec][1][64:128, sl], qc[64:128, :], qs[64:128, :])
                else:
                    nc.vector.tensor_add(qt[ec][:, sl], qc[:], qs[:])

            # ======== stage A: per-sg V + K/Q ec0 projections ========
            with tc.tile_pool(name="qap", bufs=2) as qap, \
                 tc.tile_pool(name="ropa", bufs=3) as ropa, \
                 tc.tile_pool(name="pa", bufs=4, space="PSUM") as pap, \
                 tc.tile_pool(name="pv", bufs=3, space="PSUM") as pvp, \
                 tc.tile_pool(name="wp", bufs=1, space="PSUM") as wpp:

                # warm up the PE clock-gate while input DMAs land
                nc.vector.memset(warm[:], 0.0)
                wp = wpp.tile([128, 256], f32, tag="warm", name="warm")
                for _ in range(26):
                    nc.tensor.matmul(wp[:], warm[:, 0:128], warm[:],
                                     start=True, stop=True)

                # zero pads + softmax-denominator ones (gpsimd is idle here)
                for c in range(2):
                    nc.gpsimd.memset(ktz[c][0][64:128, :], 0.0)
                    nc.gpsimd.memset(ktz[c][1][0:64, :], 0.0)
                nc.gpsimd.memset(vhv[:, :, :, 0:1], 1.0)

                kq_qa = {}
                for is_k in (True, False):
                    kq_qa[is_k] = (
                        qap.tile([128, S], f16, tag="qa", name="qa"),
                        qap.tile([128, S], f16, tag="qas", name="qas"))

                for sg in range(4):
                    sl = slice(512 * sg, 512 * (sg + 1))
                    for is_k, wsv in ((True, wkv), (False, wqv)):
                        ps = pap.tile([128, 512], f32, tag="pa", name="pa")
                        for dc in range(8):
                            nc.tensor.matmul(
                                ps[:], wsv[:, dc, 0:128], xtv[:, dc, sl],
                                start=(dc == 0), stop=(dc == 7))
                        qa, qas = kq_qa[is_k]
                        rope_chunk(ps, qa, qas, sg, is_k, 0, ropa,
                                   evac=nc.scalar.copy)
                    for i in range(4):
                        sc = 4 * sg + i
                        psv = pvp.tile([128, 256], f32, tag="pv", name="pv")
                        for dc in range(8):
                            nc.tensor.matmul(
                                psv[:],
                                xtv[:, dc, 128 * sc:128 * (sc + 1)],
                                wvv[:, dc, :],
                                start=(dc == 0), stop=(dc == 7))
                        eng = nc.scalar.copy if i % 2 else nc.vector.tensor_copy
                        eng(vhv[:, :, sc, 64:128],
                            psv[:].rearrange("p (h e) -> p h e", h=HL))

            # ======== stage B: attention (head-pair outer) + out-proj ========
            with tc.tile_pool(name="pb", bufs=1) as pb, \
                 tc.tile_pool(name="ptp", bufs=8) as ptp, \
                 tc.tile_pool(name="nrm", bufs=4) as nrmp, \
                 tc.tile_pool(name="ysb", bufs=2) as ysbp, \
                 tc.tile_pool(name="kqp", bufs=2) as kqp, \
                 tc.tile_pool(name="ropb", bufs=3) as ropb, \
                 tc.tile_pool(name="ps_s", bufs=2, space="PSUM") as ps_s, \
                 tc.tile_pool(name="ps_pv", bufs=2, space="PSUM") as ps_pv, \
                 tc.tile_pool(name="ps_y", bufs=1, space="PSUM") as ps_y, \
                 tc.tile_pool(name="ps_bc", bufs=1, space="PSUM") as ps_bc:

                aot = [pb.tile([128, S], f16, tag=f"aot{c}", name=f"aot{c}") for c in range(2)]
                ones_sb = pb.tile([1, 64], f32r, tag="ones", name="ones")
                onesf = pb.tile([1, 64], f32, tag="onesf", name="onesf")
                nc.vector.memset(onesf[:], 1.0)
                nc.vector.tensor_copy(ones_sb[:], onesf[:])

                # ec1 projection chunks, emitted as PE filler inside head-
                # pair 0's attention stream (use the out-projection's PSUM
                # slot, which is idle until head-pair 1)
                kq_tiles = {}
                for is_k in (True, False):
                    kq_tiles[is_k] = (
                        kqp.tile([128, S], f16, tag="qa", name="qa"),
                        kqp.tile([128, S], f16, tag="qas", name="qas"))

                # ec1 projection chunks as ~0.9us PE pieces: half the
                # accumulation chain per piece (other-bank matmuls may
                # interleave inside an open PSUM accumulation group)
                def mk_proj_pieces(is_k, sg):
                    box = {}

                    def piece1():
                        wsv = wkv if is_k else wqv
                        box["ps"] = ps_y.tile([128, 512], f32, tag="yp", name="yp")
                        for dc in range(4):
                            nc.tensor.matmul(
                                box["ps"][:],
                                wsv[:, dc, 128:256],
                                xtv[:, dc, 512 * sg:512 * (sg + 1)],
                                start=(dc == 0), stop=False)

                    def piece2():
                        wsv = wkv if is_k else wqv
                        qa, qas = kq_tiles[is_k]
                        for dc in range(4, 8):
                            nc.tensor.matmul(
                                box["ps"][:],
                                wsv[:, dc, 128:256],
                                xtv[:, dc, 512 * sg:512 * (sg + 1)],
                                start=False, stop=(dc == 7))
                        rope_chunk(ps=box["ps"], qa=qa, qas=qas, sg=sg,
                                   is_k=is_k, ec=1, ropool=ropb,
                                   evac=nc.vector.tensor_copy)
                    return [piece1, piece2]

                filler_q = []
                for is_k in (True, False):
                    for sg in range(4):
                        filler_q += mk_proj_pieces(is_k, sg)

                def out_proj_eg(sc, eg, ysb):
                    yp = ps_y.tile([128, 512], f32, tag="yp", name="yp")
                    for c2 in range(2):
                        nc.tensor.matmul(
                            yp[:],
                            aot[c2][:, 128 * sc:128 * (sc + 1)],
                            wov[:, c2, 512 * eg:512 * (eg + 1)],
                            start=(c2 == 0), stop=(c2 == 1))
                    nc.vector.tensor_copy(
                        ysb[:, 512 * eg:512 * (eg + 1)], yp[:])
                    if eg == 1:
                        for half in range(2):
                            sl = slice(512 * half, 512 * (half + 1))
                            nc.sync.dma_start(
                                y[128 * sc:128 * (sc + 1), sl], ysb[:, sl])

                def out_proj_pieces(sc):
                    box = {}

                    def p1():
                        box["ysb"] = ysbp.tile([128, D], f16, tag="ysb", name="ysb")
                        out_proj_eg(sc, 0, box["ysb"])

                    def p2():
                        out_proj_eg(sc, 1, box["ysb"])
                    return [p1, p2]

                pending = []   # deferred normalize closures

                def emit_pending_one():
                    if pending:
                        pending.pop(0)()

                SKEW = 3
                for hp in range(2):
                    for qg in range(4):
                        n_kc = 4 * qg + 4
                        # flush the previous stream's two normalizes early,
                        # then enqueue that q-group's out-projection pieces
                        # as per-kc PE filler
                        norm_at = {} if (hp, qg) == (0, 0) else {1: 2}
                        if hp == 1 and qg >= 1:
                            for sc in range(4 * (qg - 1), 4 * qg):
                                filler_q += out_proj_pieces(sc)
                        ppv = {}
                        for hh in range(2):
                            h = 2 * hp + hh
                            ppv[h] = ps_pv.tile([128, 512], f32, tag="ppv", name="ppv")
                        ptq = {}
                        for kc in range(n_kc + SKEW):
                            for _ in range(norm_at.get(kc, 0)):
                                emit_pending_one()
                            if kc >= 2 and filler_q:
                                filler_q.pop(0)()
                            # PV first: keeps queued work ahead of a score
                            # matmul that may block on PSUM reuse
                            kcp = kc - SKEW
                            if kcp >= 0:
                                ptv2, q0v = ptq.pop(kcp)
                                for hh in range(2):
                                    h = 2 * hp + hh
                                    nc.tensor.matmul(
                                        ppv[h][:, q0v:512],
                                        vhv[:, h, kcp, :],
                                        ptv2[:, 512 * hh + q0v:512 * (hh + 1)],
                                        start=(kcp == 0), stop=(kcp == n_kc - 1))
                            if kc < n_kc:
                                # diagonal tiles only need q >= k
                                r = kc - 4 * qg
                                q0 = 128 * r if r > 0 else 0
                                qsl = slice(512 * qg + q0, 512 * (qg + 1))
                                ps2 = ps_s.tile([128, 1024], f32, tag="ps", name="ps")
                                for hh in range(2):
                                    nc.tensor.matmul(
                                        ps2[:, 512 * hh + q0:512 * (hh + 1)],
                                        ktz[hp][hh][:, 128 * kc:128 * (kc + 1)],
                                        qt[hp][:, qsl],
                                        start=True, stop=True)
                                pt = ptp.tile([128, 1024], f16, tag="pt", name="pt")
                                psv2 = ps2[:].rearrange("p (h q) -> p h q", h=2)[:, :, q0:512]
                                ptv = pt[:].rearrange("p (h q) -> p h q", h=2)[:, :, q0:512]
                                nc.scalar.activation(
                                    ptv, psv2,
                                    mybir.ActivationFunctionType.Exp,
                                    scale=0.125)
                                if r >= 0:
                                    # only the 128-col diagonal sub-block can
                                    # have q < k; the rest is already causal
                                    for hh in range(2):
                                        nc.gpsimd.affine_select(
                                            pt[:, 512 * hh + q0:512 * hh + q0 + 128],
                                            pt[:, 512 * hh + q0:512 * hh + q0 + 128],
                                            pattern=[[1, 128]],
                                            compare_op=AluOpType.is_ge, fill=0.0,
                                            base=512 * qg + q0 - 128 * kc,
                                            channel_multiplier=-1)
                                ptq[kc] = (pt, q0)
                        # evacuate ppv fast: BOTH attn-out+denom copies first
                        # (they gate PSUM reuse), then the cheap reciprocals
                        daos = []
                        for hh in range(2):
                            h = 2 * hp + hh
                            dao = nrmp.tile([128, 512], f32, tag="dao", name="dao")
                            nc.vector.tensor_copy(dao[:], ppv[h][:])
                            daos.append(dao)
                        for hh in range(2):
                            dao = daos[hh]
                            rec = nrmp.tile([1, 512], f32, tag="rec", name="rec")
                            nc.vector.reciprocal_approx_fast(
                                rec[0:1, :], dao[0:1, :])
                            recr = nrmp.tile([1, 512], f32r, tag="recr", name="recr")
                            nc.vector.tensor_copy(recr[:], rec[:])

                            def mk_norm(qg=qg, c2=hp, off=64 * hh, rec=recr, dao=dao):
                                def emit():
                                    # PE-broadcast 1/denom across the 64 head
                                    # dims, then normalize into aot
                                    bc = ps_bc.tile([64, 512], f32, tag="bc", name="bc")
                                    nc.tensor.matmul(bc[:], ones_sb[:],
                                                     rec[:],
                                                     start=True, stop=True)
                                    nc.vector.tensor_mul(
                                        aot[c2][off:off + 64, 512 * qg:512 * (qg + 1)],
                                        dao[64:128, :], bc[:])
                                return emit
                            pending.append(mk_norm())
                # tail: the two remaining normalizes, then the last four
                # out-projection chunks.  The wide score-PSUM pool is idle
                # now -- run each chunk out of one [128,1024] tile (two yp
                # slots) so consecutive chunks never serialize on PSUM
                # reuse; split the evacuation across ACT + DVE and the
                # writeback DMAs across both queues.
                while pending:
                    emit_pending_one()
                for i in range(4):
                    sc = 12 + i
                    ps2 = ps_s.tile([128, 1024], f32, tag="ps", name="ps")
                    for eg in range(2):
                        for c2 in range(2):
                            nc.tensor.matmul(
                                ps2[:, 512 * eg:512 * (eg + 1)],
                                aot[c2][:, 128 * sc:128 * (sc + 1)],
                                wov[:, c2, 512 * eg:512 * (eg + 1)],
                                start=(c2 == 0), stop=(c2 == 1))
                    ysb = ysbp.tile([128, D], f16, tag="ysb", name="ysb")
                    nc.scalar.copy(ysb[:, 0:512], ps2[:, 0:512])
                    nc.vector.tensor_copy(ysb[:, 512:1024], ps2[:, 512:1024])
                    for half in range(2):
                        sl = slice(512 * half, 512 * (half + 1))
                        eng = nc.sync if (i + half) % 2 == 0 else nc.scalar
                        eng.dma_start(
                            y[128 * sc:128 * (sc + 1), sl], ysb[:, sl])

    nc.compile()
    return nc


def _prep_inputs(x, token_positions, Wq, Wk, Wv, Wo):
    # even/odd interleave permutation within each head (for rotate-half RoPE)
    perm = np.concatenate([np.arange(0, DK, 2), np.arange(1, DK, 2)])

    pos = np.asarray(token_positions).astype(np.float32)
    angles = THETA ** (-np.arange(32, dtype=np.float32) / 32.0)
    ang = pos[:, None] * angles[None, :]          # [S, 32]
    cos32 = np.cos(ang).T.astype(np.float32)      # [32, S]
    sin32 = np.sin(ang).T.astype(np.float32)
    cos128 = np.concatenate([cos32, cos32, cos32, cos32], axis=0)
    sin128 = np.concatenate([-sin32, sin32, -sin32, sin32], axis=0)
    cos128 = np.ascontiguousarray(cos128).astype(np.float16)
    sin128 = np.ascontiguousarray(sin128).astype(np.float16)

    Wq = np.asarray(Wq, dtype=np.float32)
    Wk = np.asarray(Wk, dtype=np.float32)
    Wv = np.asarray(Wv, dtype=np.float32)
    Wo = np.asarray(Wo, dtype=np.float32)
    x = np.asarray(x, dtype=np.float32)

    f16 = np.float16

    def pack_w(wT):
        # [1024 d, 256 e] -> [128 p, 8 dc, 256 e]
        return np.ascontiguousarray(
            wT.reshape(8, 128, EL).transpose(1, 0, 2).reshape(128, 8 * EL)
        ).astype(f16)

    in_maps = []
    for c in range(N_CORES):
        b = c // 4
        h0 = (c % 4) * HL
        esl = slice(h0 * DK, (h0 + HL) * DK)
        wq_h = Wq[esl].reshape(HL, DK, D)[:, perm].reshape(EL, D)
        wk_h = Wk[esl].reshape(HL, DK, D)[:, perm].reshape(EL, D)
        wv_h = Wv[esl]
        xT = x[b].T  # [1024 d, 2048 s]
        xt_p = np.ascontiguousarray(
            xT.reshape(8, 128, 4, 512).transpose(2, 1, 0, 3)).astype(f16)
        woT = Wo[:, esl].T  # [256 e, 1024 d_out]
        wo_p = np.ascontiguousarray(
            woT.reshape(2, 128, D).transpose(1, 0, 2).reshape(128, 2 * D)
        ).astype(f16)
        in_maps.append({
            "xt": xt_p,
            "wq": pack_w(wq_h.T),
            "wk": pack_w(wk_h.T),
            "wv": pack_w(wv_h.T),
            "wo": wo_p,
            "cosT": cos128,
            "sinT": sin128,
        })
    return in_maps


def kernel(x, token_positions, Wq, Wk, Wv, Wo, _trace=False):
    from concourse.bass_utils import run_bass_kernel_spmd

    global _compiled
    if _compiled is None:
        _compiled = _build()
    in_maps = _prep_inputs(x, token_positions, Wq, Wk, Wv, Wo)
    res = run_bass_kernel_spmd(_compiled, in_maps, list(range(N_CORES)),
                               trace=_trace)
    parts = [res.results[c]["y"].astype(np.float64) for c in range(N_CORES)]
    out = np.empty((2, S, D), dtype=np.float32)
    out[0] = (parts[0] + parts[1] + parts[2] + parts[3]).astype(np.float32)
    out[1] = (parts[4] + parts[5] + parts[6] + parts[7]).astype(np.float32)
    if _trace:
        return out, res
    return out


# revision 18
# speedup vs baseline: 1.1386x; 1.1386x over previous
"""Trainium2 Bass kernel: multi-head self-attention with RoPE, causal mask.

Reference semantics (B=2, S=2048, D=1024, H=16, DK=64):
    q = rope(x @ Wq.T), k = rope(x @ Wk.T), v = x @ Wv.T   (per-head views)
    out = softmax(causal(q k^T / 8)) v ;  y = out @ Wo.T

Sharding over 8 cores: 2-way batch x 4-way heads (4 heads/core).
Each core computes a partial y [S, D] (its heads' contribution); host sums
the 4 partials per batch (device output is fp16, summed in fp64 on host).

On-device layout strategy (per core):
  - all 16-bit operands are fp16; host prepacks every weight/input so each
    SBUF tensor loads with ONE wide DMA (xT in 4 per-sg transfers)
  - input loads ride the scalar queue; SBUF-SBUF rope swaps and y
    writebacks ride sync (no head-of-line blocking between the streams)
  - stage A is emitted per-512-column group (sg): K ec0 + Q ec0 + four V
    chunks, so the PE streams as soon as each sg's xT slice lands
  - V is projected TRANSPOSED directly (lhsT = x chunk, rhs = Wv) into
    [s, e] PSUM tiles -- no PE transpose pass; one strided copy scatters
    all 4 heads into the V working layout (ones col 0 for the softmax
    denominator, data at cols 64..127)
  - attention is HEAD-PAIR-OUTER with the ec1 K/Q projection chunks and
    the out-projection interleaved into the kc streams as PE filler
  - scores are computed TRANSPOSED (k on partitions, q on free); both
    heads of a pair share ONE two-bank PSUM tile so a single Exp covers
    both; causal masks only touch the true 128-col diagonal sub-block
  - rope multiplies all on DVE (gpsimd only does masks + memsets)
  - normalization: reciprocal_approx_fast written straight into an f32r
    tile + PE broadcast matmul; multiply deferred into the next stream
"""

import sys

sys.path.insert(0, "/opt/trn_rl_repo")

import numpy as np


S = 2048
D = 1024
NH = 16
DK = 64
HL = 4          # heads per core
EL = HL * DK    # 256 local e-dims
N_CORES = 8
THETA = 10000.0

_compiled = None


def _build():
    import concourse.bacc as bacc
    import concourse.tile as tile
    from concourse import mybir
    from concourse.alu_op_type import AluOpType

    dt = mybir.dt
    f32, f32r = dt.float32, dt.float32r
    f16 = dt.float16

    nc = bacc.Bacc("TRN2", target_bir_lowering=False, debug=False,
                   num_devices=N_CORES)

    xt_d = nc.dram_tensor("xt", [4, 128, 8, 512], f16, kind="ExternalInput").ap()
    wq_d = nc.dram_tensor("wq", [128, 8 * EL], f16, kind="ExternalInput").ap()
    wk_d = nc.dram_tensor("wk", [128, 8 * EL], f16, kind="ExternalInput").ap()
    wv_d = nc.dram_tensor("wv", [128, 8 * EL], f16, kind="ExternalInput").ap()
    wo_d = nc.dram_tensor("wo", [128, 2 * D], f16, kind="ExternalInput").ap()
    cos_d = nc.dram_tensor("cosT", [128, S], f16, kind="ExternalInput").ap()
    sin_d = nc.dram_tensor("sinT", [128, S], f16, kind="ExternalInput").ap()
    sel_d = nc.dram_tensor("sel2", [1, 256], f32r, kind="ExternalInput").ap()
    y = nc.dram_tensor("y", [S, D], f16, kind="ExternalOutput").ap()

    with tile.TileContext(nc) as tc:
        with tc.tile_pool(name="persist", bufs=1) as pp:
            # persistent SBUF tiles (live across both stages)
            qt = [pp.tile([128, S], f16, tag=f"qt{c}", name=f"qt{c}") for c in range(2)]
            ktz = [[pp.tile([128, S], f16, tag=f"ktz{c}{par}", name=f"ktz{c}{par}")
                   for par in range(2)] for c in range(2)]
            vh_all = pp.tile([128, HL * 16 * 128], f16, tag="vh", name="vh")
            cos_sb = pp.tile([128, S], f16, tag="cos", name="cos")
            sin_sb = pp.tile([128, S], f16, tag="sin", name="sin")
            xt_all = pp.tile([128, 8 * S], f16, tag="xt", name="xt")
            wv_all = pp.tile([128, 8 * EL], f16, tag="wv", name="wv")
            wk_all = pp.tile([128, 8 * EL], f16, tag="wk", name="wk")
            wq_all = pp.tile([128, 8 * EL], f16, tag="wq", name="wq")
            wo_all = pp.tile([128, 2 * D], f16, tag="wo", name="wo")
            warm = pp.tile([128, 256], f16, tag="warm", name="warm")

            xtv = xt_all[:].rearrange("p (d s) -> p d s", d=8)
            wvv = wv_all[:].rearrange("p (d e) -> p d e", d=8)
            wkv = wk_all[:].rearrange("p (d e) -> p d e", d=8)
            wqv = wq_all[:].rearrange("p (d e) -> p d e", d=8)
            wov = wo_all[:].rearrange("p (c d) -> p c d", c=2)
            vhv = vh_all[:].rearrange("p (h s c) -> p h s c", h=HL, c=128)

            # ---- input DMA program ----
            # scalar queue: ALL HBM loads in arrival-priority order (one
            # queue with wide transfers saturates HBM; cos/sin last -- the
            # rope multiplies that need them run on DVE well after the
            # projections).  sync queue: only SBUF-SBUF swaps + y writes.
            nc.scalar.dma_start(wv_all[:], wv_d[:])
            nc.scalar.dma_start(xtv[:, :, 0:512], xt_d[0])
            nc.scalar.dma_start(wk_all[:], wk_d[:])
            nc.scalar.dma_start(xtv[:, :, 512:1024], xt_d[1])
            nc.scalar.dma_start(wq_all[:], wq_d[:])
            nc.scalar.dma_start(xtv[:, :, 1024:1536], xt_d[2])
            nc.scalar.dma_start(xtv[:, :, 1536:2048], xt_d[3])
            nc.scalar.dma_start(wo_all[:], wo_d[:])
            nc.scalar.dma_start(cos_sb[:], cos_d[:])
            nc.scalar.dma_start(sin_sb[:], sin_d[:])

            # rope chunk: evacuate PSUM proj, SBUF-to-SBUF DMA block swap to
            # build the rotate-half partner, cos/sin multiplies + add (DVE)
            def rope_chunk(ps, qa, qas, sg, is_k, ec, ropool, evac):
                sl = slice(512 * sg, 512 * (sg + 1))
                evac(qa[:, sl], ps[:])
                for blk in range(2):
                    b0 = 64 * blk
                    nc.sync.dma_start(
                        qas[b0:b0 + 32, sl], qa[b0 + 32:b0 + 64, sl])
                    nc.sync.dma_start(
                        qas[b0 + 32:b0 + 64, sl], qa[b0:b0 + 32, sl])
                qc = ropool.tile([128, 512], f16, tag="qc", name="qc")
                qs = ropool.tile([128, 512], f16, tag="qs", name="qs")
                nc.vector.tensor_mul(qc[:], qa[:, sl], cos_sb[:, sl])
                nc.gpsimd.tensor_mul(qs[:], qas[:, sl], sin_sb[:, sl])
                if is_k:
                    # zero-padded K halves so score matmuls see K=128 rows
                    nc.vector.tensor_add(
                        ktz[ec][0][0:64, sl], qc[0:64, :], qs[0:64, :])
                    nc.vector.tensor_add(
                        ktz[ec][1][64:128, sl], qc[64:128, :], qs[64:128, :])
                else:
                    nc.vector.tensor_add(qt[ec][:, sl], qc[:], qs[:])

            # ======== stage A: per-sg V + K/Q ec0 projections ========
            with tc.tile_pool(name="qap", bufs=2) as qap, \
                 tc.tile_pool(name="ropa", bufs=3) as ropa, \
                 tc.tile_pool(name="pa", bufs=4, space="PSUM") as pap, \
                 tc.tile_pool(name="pv", bufs=3, space="PSUM") as pvp, \
                 tc.tile_pool(name="wp", bufs=1, space="PSUM") as wpp:

                # warm up the PE clock-gate while input DMAs land
                nc.vector.memset(warm[:], 0.0)
                wp = wpp.tile([128, 256], f32, tag="warm", name="warm")
                for _ in range(18):
                    nc.tensor.matmul(wp[:], warm[:, 0:128], warm[:],
                                     start=True, stop=True)

                # zero pads + softmax-denominator ones (gpsimd is idle here)
                for c in range(2):
                    nc.gpsimd.memset(ktz[c][0][64:128, :], 0.0)
                    nc.gpsimd.memset(ktz[c][1][0:64, :], 0.0)
                nc.gpsimd.memset(vhv[:, :, :, 0:1], 1.0)

                kq_qa = {}
                for is_k in (True, False):
                    kq_qa[is_k] = (
                        qap.tile([128, S], f16, tag="qa", name="qa"),
                        qap.tile([128, S], f16, tag="qas", name="qas"))

                def kq_chunk(sg, is_k, wsv):
                    sl = slice(512 * sg, 512 * (sg + 1))
                    ps = pap.tile([128, 512], f32, tag="pa", name="pa")
                    for dc in range(8):
                        nc.tensor.matmul(
                            ps[:], wsv[:, dc, 0:128], xtv[:, dc, sl],
                            start=(dc == 0), stop=(dc == 7))
                    qa, qas = kq_qa[is_k]
                    rope_chunk(ps, qa, qas, sg, is_k, 0, ropa,
                               evac=nc.scalar.copy)

                def v_chunk(sc):
                    psv = pvp.tile([128, 256], f32, tag="pv", name="pv")
                    for dc in range(8):
                        nc.tensor.matmul(
                            psv[:],
                            xtv[:, dc, 128 * sc:128 * (sc + 1)],
                            wvv[:, dc, :],
                            start=(dc == 0), stop=(dc == 7))
                    eng = nc.scalar.copy if sc % 2 else nc.vector.tensor_copy
                    eng(vhv[:, :, sc, 64:128],
                        psv[:].rearrange("p (h e) -> p h e", h=HL))

                for sg in range(4):
                    if sg == 0:
                        # sg0: V first -- wv+xt0 land before wk
                        for i in range(4):
                            v_chunk(i)
                        kq_chunk(0, True, wkv)
                        kq_chunk(0, False, wqv)
                    else:
                        kq_chunk(sg, True, wkv)
                        kq_chunk(sg, False, wqv)
                        for i in range(4):
                            v_chunk(4 * sg + i)

            # ======== stage B: attention (head-pair outer) + out-proj ========
            with tc.tile_pool(name="pb", bufs=1) as pb, \
                 tc.tile_pool(name="ptp", bufs=8) as ptp, \
                 tc.tile_pool(name="nrm", bufs=4) as nrmp, \
                 tc.tile_pool(name="ysb", bufs=2) as ysbp, \
                 tc.tile_pool(name="kqp", bufs=2) as kqp, \
                 tc.tile_pool(name="ropb", bufs=3) as ropb, \
                 tc.tile_pool(name="ps_s", bufs=2, space="PSUM") as ps_s, \
                 tc.tile_pool(name="ps_pv", bufs=2, space="PSUM") as ps_pv, \
                 tc.tile_pool(name="ps_y", bufs=1, space="PSUM") as ps_y, \
                 tc.tile_pool(name="ps_bc", bufs=1, space="PSUM") as ps_bc:

                aot = [pb.tile([128, S], f16, tag=f"aot{c}", name=f"aot{c}") for c in range(2)]
                # head-pair selectors for the 1/denom broadcast: sel2 col
                # block hh holds ones on partitions [64*hh, 64*hh+64)
                sel2 = pb.tile([1, 256], f32r, tag="sel2", name="sel2")
                nc.scalar.dma_start(sel2[:], sel_d[:])

                # ec1 projection chunks, emitted as PE filler inside head-
                # pair 0's attention stream (use the out-projection's PSUM
                # slot, which is idle until head-pair 1)
                kq_tiles = {}
                for is_k in (True, False):
                    kq_tiles[is_k] = (
                        kqp.tile([128, S], f16, tag="qa", name="qa"),
                        kqp.tile([128, S], f16, tag="qas", name="qas"))

                # ec1 projection chunks as ~0.9us PE pieces: half the
                # accumulation chain per piece (other-bank matmuls may
                # interleave inside an open PSUM accumulation group)
                def mk_proj_pieces(is_k, sg):
                    box = {}

                    def piece1():
                        wsv = wkv if is_k else wqv
                        box["ps"] = ps_y.tile([128, 512], f32, tag="yp", name="yp")
                        for dc in range(4):
                            nc.tensor.matmul(
                                box["ps"][:],
                                wsv[:, dc, 128:256],
                                xtv[:, dc, 512 * sg:512 * (sg + 1)],
                                start=(dc == 0), stop=False)

                    def piece2():
                        wsv = wkv if is_k else wqv
                        qa, qas = kq_tiles[is_k]
                        for dc in range(4, 8):
                            nc.tensor.matmul(
                                box["ps"][:],
                                wsv[:, dc, 128:256],
                                xtv[:, dc, 512 * sg:512 * (sg + 1)],
                                start=False, stop=(dc == 7))
                        rope_chunk(ps=box["ps"], qa=qa, qas=qas, sg=sg,
                                   is_k=is_k, ec=1, ropool=ropb,
                                   evac=nc.vector.tensor_copy)
                    return [piece1, piece2]

                filler_q = []
                for is_k in (True, False):
                    for sg in range(4):
                        filler_q += mk_proj_pieces(is_k, sg)

                def out_proj_eg(sc, eg, ysb):
                    yp = ps_y.tile([128, 512], f32, tag="yp", name="yp")
                    for c2 in range(2):
                        nc.tensor.matmul(
                            yp[:],
                            aot[c2][:, 128 * sc:128 * (sc + 1)],
                            wov[:, c2, 512 * eg:512 * (eg + 1)],
                            start=(c2 == 0), stop=(c2 == 1))
                    nc.vector.tensor_copy(
                        ysb[:, 512 * eg:512 * (eg + 1)], yp[:])
                    if eg == 1:
                        for half in range(2):
                            sl = slice(512 * half, 512 * (half + 1))
                            nc.sync.dma_start(
                                y[128 * sc:128 * (sc + 1), sl], ysb[:, sl])

                def out_proj_pieces(sc):
                    box = {}

                    def p1():
                        box["ysb"] = ysbp.tile([128, D], f16, tag="ysb", name="ysb")
                        out_proj_eg(sc, 0, box["ysb"])

                    def p2():
                        out_proj_eg(sc, 1, box["ysb"])
                    return [p1, p2]

                pending = []   # deferred normalize closures

                def emit_pending_one():
                    if pending:
                        pending.pop(0)()

                SKEW = 3
                for hp in range(2):
                    for qg in range(4):
                        n_kc = 4 * qg + 4
                        # flush the previous stream's normalize early, then
                        # enqueue that q-group's out-projection pieces as
                        # per-kc PE filler
                        norm_at = {} if (hp, qg) == (0, 0) else {1: 1}
                        if hp == 1 and qg >= 1:
                            for sc in range(4 * (qg - 1), 4 * qg):
                                filler_q += out_proj_pieces(sc)
                        ppv = {}
                        for hh in range(2):
                            h = 2 * hp + hh
                            ppv[h] = ps_pv.tile([128, 512], f32, tag="ppv", name="ppv")
                        ptq = {}
                        for kc in range(n_kc + SKEW):
                            for _ in range(norm_at.get(kc, 0)):
                                emit_pending_one()
                            if kc >= 2 and filler_q:
                                filler_q.pop(0)()
                            # PV first: keeps queued work ahead of a score
                            # matmul that may block on PSUM reuse
                            kcp = kc - SKEW
                            if kcp >= 0:
                                ptv2, q0v = ptq.pop(kcp)
                                for hh in range(2):
                                    h = 2 * hp + hh
                                    nc.tensor.matmul(
                                        ppv[h][:, q0v:512],
                                        vhv[:, h, kcp, :],
                                        ptv2[:, 512 * hh + q0v:512 * (hh + 1)],
                                        start=(kcp == 0), stop=(kcp == n_kc - 1))
                            if kc < n_kc:
                                # diagonal tiles only need q >= k
                                r = kc - 4 * qg
                                q0 = 128 * r if r > 0 else 0
                                qsl = slice(512 * qg + q0, 512 * (qg + 1))
                                ps2 = ps_s.tile([128, 1024], f32, tag="ps", name="ps")
                                for hh in range(2):
                                    nc.tensor.matmul(
                                        ps2[:, 512 * hh + q0:512 * (hh + 1)],
                                        ktz[hp][hh][:, 128 * kc:128 * (kc + 1)],
                                        qt[hp][:, qsl],
                                        start=True, stop=True)
                                pt = ptp.tile([128, 1024], f16, tag="pt", name="pt")
                                psv2 = ps2[:].rearrange("p (h q) -> p h q", h=2)[:, :, q0:512]
                                ptv = pt[:].rearrange("p (h q) -> p h q", h=2)[:, :, q0:512]
                                nc.scalar.activation(
                                    ptv, psv2,
                                    mybir.ActivationFunctionType.Exp,
                                    scale=0.125)
                                if r >= 0:
                                    # only the 128-col diagonal sub-block can
                                    # have q < k; the rest is already causal
                                    for hh in range(2):
                                        nc.gpsimd.affine_select(
                                            pt[:, 512 * hh + q0:512 * hh + q0 + 128],
                                            pt[:, 512 * hh + q0:512 * hh + q0 + 128],
                                            pattern=[[1, 128]],
                                            compare_op=AluOpType.is_ge, fill=0.0,
                                            base=512 * qg + q0 - 128 * kc,
                                            channel_multiplier=-1)
                                ptq[kc] = (pt, q0)
                        # evacuate ppv fast: BOTH attn-out+denom copies first
                        # (they gate PSUM reuse), then the cheap reciprocals
                        daos = []
                        for hh in range(2):
                            h = 2 * hp + hh
                            dao = nrmp.tile([128, 512], f32, tag="dao", name="dao")
                            nc.vector.tensor_copy(dao[:], ppv[h][:])
                            daos.append(dao)
                        recf = nrmp.tile([1, 1024], f32, tag="rec", name="rec")
                        for hh in range(2):
                            nc.vector.reciprocal_approx_fast(
                                recf[0:1, 512 * hh:512 * (hh + 1)],
                                daos[hh][0:1, :])
                        recr = nrmp.tile([1, 1024], f32r, tag="recr", name="recr")
                        nc.vector.tensor_copy(recr[:], recf[:])

                        def mk_norm(qg=qg, c2=hp, rec=recr, daos=daos):
                            def emit():
                                # two accumulating matmuls broadcast BOTH
                                # heads' 1/denom into one bc bank, then
                                # normalize into aot
                                bc = ps_bc.tile([128, 512], f32, tag="bc", name="bc")
                                for hh in range(2):
                                    nc.tensor.matmul(
                                        bc[:],
                                        sel2[0:1, 128 * hh:128 * (hh + 1)],
                                        rec[0:1, 512 * hh:512 * (hh + 1)],
                                        start=(hh == 0), stop=(hh == 1))
                                for hh in range(2):
                                    nc.vector.tensor_mul(
                                        aot[c2][64 * hh:64 * hh + 64,
                                                512 * qg:512 * (qg + 1)],
                                        daos[hh][64:128, :],
                                        bc[64 * hh:64 * hh + 64, :])
                            return emit
                        pending.append(mk_norm())
                # tail: the two remaining normalizes, then the last four
                # out-projection chunks.  The wide score-PSUM pool is idle
                # now -- run each chunk out of one [128,1024] tile (two yp
                # slots) so consecutive chunks never serialize on PSUM
                # reuse; split the evacuation across ACT + DVE and the
                # writeback DMAs across both queues.
                while pending:
                    emit_pending_one()
                for i in range(4):
                    sc = 12 + i
                    ps2 = ps_s.tile([128, 1024], f32, tag="ps", name="ps")
                    for eg in range(2):
                        for c2 in range(2):
                            nc.tensor.matmul(
                                ps2[:, 512 * eg:512 * (eg + 1)],
                                aot[c2][:, 128 * sc:128 * (sc + 1)],
                                wov[:, c2, 512 * eg:512 * (eg + 1)],
                                start=(c2 == 0), stop=(c2 == 1))
                    ysb = ysbp.tile([128, D], f16, tag="ysb", name="ysb")
                    nc.scalar.copy(ysb[:, 0:512], ps2[:, 0:512])
                    nc.vector.tensor_copy(ysb[:, 512:1024], ps2[:, 512:1024])
                    for half in range(2):
                        sl = slice(512 * half, 512 * (half + 1))
                        nc.sync.dma_start(
                            y[128 * sc:128 * (sc + 1), sl], ysb[:, sl])

    nc.compile()
    return nc


def _prep_inputs(x, token_positions, Wq, Wk, Wv, Wo):
    # even/odd interleave permutation within each head (for rotate-half RoPE)
    perm = np.concatenate([np.arange(0, DK, 2), np.arange(1, DK, 2)])

    pos = np.asarray(token_positions).astype(np.float32)
    angles = THETA ** (-np.arange(32, dtype=np.float32) / 32.0)
    ang = pos[:, None] * angles[None, :]          # [S, 32]
    cos32 = np.cos(ang).T.astype(np.float32)      # [32, S]
    sin32 = np.sin(ang).T.astype(np.float32)
    cos128 = np.concatenate([cos32, cos32, cos32, cos32], axis=0)
    sin128 = np.concatenate([-sin32, sin32, -sin32, sin32], axis=0)
    cos128 = np.ascontiguousarray(cos128).astype(np.float16)
    sin128 = np.ascontiguousarray(sin128).astype(np.float16)

    Wq = np.asarray(Wq, dtype=np.float32)
    Wk = np.asarray(Wk, dtype=np.float32)
    Wv = np.asarray(Wv, dtype=np.float32)
    Wo = np.asarray(Wo, dtype=np.float32)
    x = np.asarray(x, dtype=np.float32)

    f16 = np.float16

    def pack_w(wT):
        # [1024 d, 256 e] -> [128 p, 8 dc, 256 e]
        return np.ascontiguousarray(
            wT.reshape(8, 128, EL).transpose(1, 0, 2).reshape(128, 8 * EL)
        ).astype(f16)

    in_maps = []
    for c in range(N_CORES):
        b = c // 4
        h0 = (c % 4) * HL
        esl = slice(h0 * DK, (h0 + HL) * DK)
        wq_h = Wq[esl].reshape(HL, DK, D)[:, perm].reshape(EL, D)
        wk_h = Wk[esl].reshape(HL, DK, D)[:, perm].reshape(EL, D)
        wv_h = Wv[esl]
        xT = x[b].T  # [1024 d, 2048 s]
        xt_p = np.ascontiguousarray(
            xT.reshape(8, 128, 4, 512).transpose(2, 1, 0, 3)).astype(f16)
        woT = Wo[:, esl].T  # [256 e, 1024 d_out]
        wo_p = np.ascontiguousarray(
            woT.reshape(2, 128, D).transpose(1, 0, 2).reshape(128, 2 * D)
        ).astype(f16)
        sel2 = np.zeros((1, 256), dtype=np.float32)
        sel2[0, 0:64] = 1.0
        sel2[0, 192:256] = 1.0
        in_maps.append({
            "xt": xt_p,
            "wq": pack_w(wq_h.T),
            "wk": pack_w(wk_h.T),
            "wv": pack_w(wv_h.T),
            "wo": wo_p,
            "cosT": cos128,
            "sinT": sin128,
            "sel2": sel2,
        })
    return in_maps


def kernel(x, token_positions, Wq, Wk, Wv, Wo, _trace=False):
    from concourse.bass_utils import run_bass_kernel_spmd

    global _compiled
    if _compiled is None:
        _compiled = _build()
    in_maps = _prep_inputs(x, token_positions, Wq, Wk, Wv, Wo)
    res = run_bass_kernel_spmd(_compiled, in_maps, list(range(N_CORES)),
                               trace=_trace)
    parts = [res.results[c]["y"].astype(np.float64) for c in range(N_CORES)]
    out = np.empty((2, S, D), dtype=np.float32)
    out[0] = (parts[0] + parts[1] + parts[2] + parts[3]).astype(np.float32)
    out[1] = (parts[4] + parts[5] + parts[6] + parts[7]).astype(np.float32)
    if _trace:
        return out, res
    return out
